# revision 5
# baseline (speedup 1.0000x reference)
"""Multi-head attention (B=2, S=2048, D=1024, H=16) on 8 TRN2 NeuronCores.

Sharding: batch x head-group. Core c handles batch b=c//4 and heads
[4g, 4g+4) with g=c%4 (column-parallel QKV projections, row-parallel
output projection). Each core emits a partial [S, D] output; the host
sums the 4 partials per batch (the row-parallel all-reduce).

Device-side dataflow per core (all matmuls bf16 with f32 PSUM accum):
  qhT/khT [p=256, s] = Wg.T-weighted projections of q/k (q pre-scaled
  by 1/sqrt(hd) on host); vh [s, p] likewise, augmented with a ones
  column per head so the attention row-sums fall out of the AV matmul.
  Scores are computed transposed (scoresT[j, i]) so softmax renormali-
  zation and the AV contraction both run without any on-chip transpose:
  exp via ScalarE straight out of PSUM, causal masking via a single
  [128,128] additive bias tile on the diagonal blocks, and strictly-
  upper blocks are never computed.
"""

import os
import numpy as np
import ml_dtypes

import concourse.bass as bass
import concourse.tile as tile
from concourse import bacc, mybir
from concourse.bass_utils import run_bass_kernel_spmd

B, S, D, H = 2, 2048, 1024, 16
HD = D // H          # 64
HL = H // 4          # 4 heads per core
PL = HL * HD         # 256 local projection dim
KT = D // 128        # 8 contraction blocks
SB = S // 128        # 16 sequence blocks of 128
CH = S // 512        # 4 sequence chunks of 512
F32 = mybir.dt.float32
DT = mybir.dt.bfloat16
NP_DT = ml_dtypes.bfloat16

_cache = {}
last_results = None


def build_program():
    if "nc" in _cache:
        return _cache["nc"]
    nc = bacc.Bacc("TRN2", target_bir_lowering=False, debug=False, num_devices=8)

    qt_d = nc.dram_tensor("qt", [D, S], DT, kind="ExternalInput")
    kt_d = nc.dram_tensor("kt", [D, S], DT, kind="ExternalInput")
    vt_d = nc.dram_tensor("vt", [D, S], DT, kind="ExternalInput")
    wq_d = nc.dram_tensor("wq", [D, PL], DT, kind="ExternalInput")
    wk_d = nc.dram_tensor("wk", [D, PL], DT, kind="ExternalInput")
    wv_d = nc.dram_tensor("wv", [D, PL], DT, kind="ExternalInput")
    wf_d = nc.dram_tensor("wf", [PL, D], DT, kind="ExternalInput")
    bq_d = nc.dram_tensor("bq2", [2, 128, 1], F32, kind="ExternalInput")
    bk_d = nc.dram_tensor("bk2", [2, 128, 1], F32, kind="ExternalInput")
    bv_d = nc.dram_tensor("bv1", [1, PL], F32, kind="ExternalInput")
    bf_d = nc.dram_tensor("bf1", [1, D], F32, kind="ExternalInput")
    tri_d = nc.dram_tensor("tri", [128, 128], F32, kind="ExternalInput")
    out_d = nc.dram_tensor("out", [S, D], F32, kind="ExternalOutput")

    ADD = mybir.AluOpType.add
    MUL = mybir.AluOpType.mult
    EXP = mybir.ActivationFunctionType.Exp

    with tile.TileContext(nc) as tc:
        with (
            tc.tile_pool(name="singles", bufs=1) as singles,
            tc.tile_pool(name="inp", bufs=12) as inp,
            tc.tile_pool(name="epool", bufs=6) as epool,
            tc.tile_pool(name="apool", bufs=2) as apool,
            tc.tile_pool(name="opool", bufs=3) as opool,
            tc.tile_pool(name="psum", bufs=2, space="PSUM") as psum,
        ):
            wq_sb = singles.tile([128, KT, PL], DT)
            wk_sb = singles.tile([128, KT, PL], DT)
            wv_sb = singles.tile([128, KT, PL], DT)
            wf_sb = singles.tile([128, 2, D], DT)
            nc.sync.dma_start(wq_sb, wq_d.ap().rearrange("(k p) n -> p k n", p=128))
            nc.sync.dma_start(wk_sb, wk_d.ap().rearrange("(k p) n -> p k n", p=128))
            nc.sync.dma_start(wv_sb, wv_d.ap().rearrange("(k p) n -> p k n", p=128))
            nc.sync.dma_start(wf_sb, wf_d.ap().rearrange("(t p) n -> p t n", p=128))

            bq_sb = singles.tile([128, 2], F32)
            bk_sb = singles.tile([128, 2], F32)
            nc.sync.dma_start(bq_sb, bq_d.ap().rearrange("t p o -> p (t o)"))
            nc.sync.dma_start(bk_sb, bk_d.ap().rearrange("t p o -> p (t o)"))
            bv_bc = singles.tile([128, PL], F32)
            bf_bc = singles.tile([128, D], F32)
            nc.sync.dma_start(bv_bc, bv_d.ap().to_broadcast([128, PL]))
            nc.sync.dma_start(bf_bc, bf_d.ap().to_broadcast([128, D]))
            tri_sb = singles.tile([128, 128], F32)
            nc.sync.dma_start(tri_sb, tri_d.ap())

            qhT = singles.tile([128, 2, S], DT)   # [p within block, pblock, s]
            khT = singles.tile([128, 2, S], DT)
            vh = singles.tile([128, SB, HL, HD + 1], DT)  # [s within blk, sblk, h, hd|ones]
            xn = singles.tile([128, 2, S], DT)    # normalized attn out, head pairs stacked
            nc.vector.memset(vh[:, :, :, HD : HD + 1], 1.0)
            ones_sb = singles.tile([128, HD], F32)
            nc.vector.memset(ones_sb, 1.0)

            # ---- Phase B: projections ----
            def load_blocks(x_d):
                ts = []
                for kk in range(KT):
                    t = inp.tile([128, S], DT, tag="xin")
                    nc.sync.dma_start(t, x_d.ap()[128 * kk : 128 * (kk + 1), :])
                    ts.append(t)
                return ts

            def proj_ph(xt, w_sb, b_sb, out_sb):
                for pt in range(2):
                    for ch in range(CH):
                        pp = psum.tile([128, 512], F32, tag="A", bufs=4)
                        for kk in range(KT):
                            nc.tensor.matmul(
                                pp,
                                lhsT=w_sb[:, kk, 128 * pt : 128 * (pt + 1)],
                                rhs=xt[kk][:, 512 * ch : 512 * (ch + 1)],
                                start=(kk == 0),
                                stop=(kk == KT - 1),
                            )
                        nc.vector.tensor_scalar_add(
                            out_sb[:, pt, 512 * ch : 512 * (ch + 1)],
                            pp,
                            b_sb[:, pt : pt + 1],
                        )

            qt_t = load_blocks(qt_d)
            proj_ph(qt_t, wq_sb, bq_sb, qhT)
            kt_t = load_blocks(kt_d)
            proj_ph(kt_t, wk_sb, bk_sb, khT)
            vt_t = load_blocks(vt_d)
            for sb in range(SB):
                pv = psum.tile([128, PL], F32, tag="C", bufs=2)
                for kk in range(KT):
                    nc.tensor.matmul(
                        pv,
                        lhsT=vt_t[kk][:, 128 * sb : 128 * (sb + 1)],
                        rhs=wv_sb[:, kk, :],
                        start=(kk == 0),
                        stop=(kk == KT - 1),
                    )
                nc.vector.tensor_tensor(
                    out=vh[:, sb, :, 0:HD],
                    in0=pv.rearrange("p (h e) -> p h e", h=HL),
                    in1=bv_bc.rearrange("p (h e) -> p h e", h=HL),
                    op=ADD,
                )

            # ---- Phase C: attention per head ----
            for h in range(HL):
                ro = 64 * (h % 2)
                hb = h // 2
                for c in range(CH):
                    px = psum.tile([128, 512], F32, tag="B", bufs=2)
                    nbj = 4 * c + 4
                    for bj in range(nbj):
                        band = bj >= 4 * c
                        i0 = 128 * bj if band else 512 * c
                        w = 512 * (c + 1) - i0
                        o = i0 - 512 * c
                        ps = psum.tile([128, 512], F32, tag="A", bufs=4)
                        nc.tensor.matmul(
                            ps[:, 0:w],
                            lhsT=khT[ro : ro + 64, hb, 128 * bj : 128 * (bj + 1)],
                            rhs=qhT[ro : ro + 64, hb, i0 : i0 + w],
                            start=True,
                            stop=True,
                        )
                        if band:
                            nc.vector.tensor_tensor(
                                out=ps[:, 0:128], in0=ps[:, 0:128], in1=tri_sb, op=ADD
                            )
                        et = epool.tile([128, 512], DT, tag="et")
                        nc.scalar.activation(et[:, 0:w], ps[:, 0:w], EXP)
                        nc.tensor.matmul(
                            px[0 : HD + 1, o : o + w],
                            lhsT=vh[:, bj, h, :],
                            rhs=et[:, 0:w],
                            start=(bj == 0),
                            stop=(bj == nbj - 1),
                        )
                    xa = apool.tile([HD + 1, 512], F32, tag="xa")
                    nc.vector.tensor_copy(out=xa, in_=px[0 : HD + 1, :])
                    nc.vector.reciprocal(xa[HD : HD + 1, :], xa[HD : HD + 1, :])
                    # broadcast the reciprocal row across 64 partitions via a
                    # K=1 fp32 matmul (gpsimd partition_broadcast is broken on
                    # HW; step-0-partition DMA from SBUF is rejected)
                    rb = psum.tile([HD, 512], F32, tag="C", bufs=2)
                    nc.tensor.matmul(
                        rb,
                        lhsT=ones_sb[HD : HD + 1, :],
                        rhs=xa[HD : HD + 1, :],
                        start=True,
                        stop=True,
                    )
                    xt_n = apool.tile([HD, 512], DT, tag="xtn")
                    nc.vector.tensor_tensor(out=xt_n, in0=xa[0:HD, :], in1=rb, op=MUL)
                    nc.sync.dma_start(
                        xn[ro : ro + 64, hb, 512 * c : 512 * (c + 1)], xt_n
                    )

            # ---- Phase D: output projection (partial; host sums over groups) ----
            for ib in range(SB):
                for oc in range(2):
                    po = psum.tile([128, 512], F32, tag="A", bufs=4)
                    for t in range(2):
                        nc.tensor.matmul(
                            po,
                            lhsT=xn[:, t, 128 * ib : 128 * (ib + 1)],
                            rhs=wf_sb[:, t, 512 * oc : 512 * (oc + 1)],
                            start=(t == 0),
                            stop=(t == 1),
                        )
                    ob = opool.tile([128, 512], F32, tag="ob")
                    nc.vector.tensor_tensor(
                        out=ob, in0=po, in1=bf_bc[:, 512 * oc : 512 * (oc + 1)], op=ADD
                    )
                    nc.sync.dma_start(
                        out_d.ap()[128 * ib : 128 * (ib + 1), 512 * oc : 512 * (oc + 1)],
                        ob,
                    )

    nc.compile()
    _cache["nc"] = nc
    return nc


def make_in_maps(q, k, v, mask, Wq, bq, Wk, bk, Wv, bv, Wf, bf):
    scale = 1.0 / np.sqrt(np.float32(HD))
    f32 = np.float32
    m = np.asarray(mask[0, 0])
    tri = np.where(m[:128, :128].T == 0, f32(-1e9), f32(0.0)).astype(f32)
    in_maps = []
    for c in range(8):
        b, g = c // 4, c % 4
        sl = slice(g * PL, (g + 1) * PL)
        in_maps.append(
            {
                "qt": np.ascontiguousarray((np.asarray(q[b]).T * scale)).astype(NP_DT),
                "kt": np.ascontiguousarray(np.asarray(k[b]).T).astype(NP_DT),
                "vt": np.ascontiguousarray(np.asarray(v[b]).T).astype(NP_DT),
                "wq": np.ascontiguousarray(np.asarray(Wq)[sl, :].T).astype(NP_DT),
                "wk": np.ascontiguousarray(np.asarray(Wk)[sl, :].T).astype(NP_DT),
                "wv": np.ascontiguousarray(np.asarray(Wv)[sl, :].T).astype(NP_DT),
                "wf": np.ascontiguousarray(np.asarray(Wf)[:, sl].T).astype(NP_DT),
                "bq2": (np.asarray(bq)[sl] * scale).astype(f32).reshape(2, 128, 1),
                "bk2": np.asarray(bk)[sl].astype(f32).reshape(2, 128, 1),
                "bv1": np.asarray(bv)[sl].astype(f32).reshape(1, PL),
                "bf1": (np.asarray(bf).astype(f32) / 4.0).reshape(1, D),
                "tri": tri,
            }
        )
    return in_maps


def _mask_is_causal(mask):
    m = np.asarray(mask[0, 0])
    return bool(np.array_equal(m != 0, np.tril(np.ones((S, S), bool))))


def _numpy_fallback(q, k, v, mask, Wq, bq, Wk, bk, Wv, bv, Wf, bf):
    out = np.empty((B, S, D), np.float32)
    m = np.asarray(mask[0, 0])
    for b in range(B):
        qh = (np.asarray(q[b]) @ np.asarray(Wq).T + bq).reshape(S, H, HD)
        kh = (np.asarray(k[b]) @ np.asarray(Wk).T + bk).reshape(S, H, HD)
        vh = (np.asarray(v[b]) @ np.asarray(Wv).T + bv).reshape(S, H, HD)
        x = np.empty((S, H, HD), np.float32)
        for hh in range(H):
            sc = qh[:, hh] @ kh[:, hh].T / np.sqrt(np.float32(HD))
            sc = np.where(m == 0, np.float32(-1e9), sc)
            sc = sc - sc.max(-1, keepdims=True)
            e = np.exp(sc)
            x[:, hh] = (e / e.sum(-1, keepdims=True)) @ vh[:, hh]
        out[b] = x.reshape(S, D) @ np.asarray(Wf).T + bf
    return out


def kernel(q, k, v, mask, Wq, bq, Wk, bk, Wv, bv, Wf, bf):
    global last_results
    if not _mask_is_causal(mask):
        return _numpy_fallback(q, k, v, mask, Wq, bq, Wk, bk, Wv, bv, Wf, bf)
    nc = build_program()
    in_maps = make_in_maps(q, k, v, mask, Wq, bq, Wk, bk, Wv, bv, Wf, bf)
    res = run_bass_kernel_spmd(nc, in_maps, core_ids=list(range(8)))
    last_results = res
    out = np.zeros((B, S, D), np.float32)
    for c in range(8):
        out[c // 4] += res.results[c]["out"]
    return out


# revision 12
# speedup vs baseline: 1.3126x; 1.3126x over previous
"""Multi-head attention (B=2, S=2048, D=1024, H=16) on 8 TRN2 NeuronCores.

Sharding: batch x head-group. Core c handles batch b=c//4 and heads
[4g, 4g+4) with g=c%4 (column-parallel QKV projections, row-parallel
output projection). Each core emits a partial [S, D] output; the host
sums the 4 partials per batch (the row-parallel all-reduce).

Device-side dataflow per core (all matmuls bf16 with f32 PSUM accum):
  qhT/khT [p=256, s] = Wg.T-weighted projections of q/k (q pre-scaled
  by 1/sqrt(hd) on host); vh [s, p] likewise, augmented with a ones
  column per head so the attention row-sums fall out of the AV matmul.
  Scores are computed transposed (scoresT[j, i]) so softmax renormali-
  zation and the AV contraction both run without any on-chip transpose:
  exp via ScalarE straight out of PSUM, causal masking via a single
  [128,128] additive bias tile on the diagonal blocks, and strictly-
  upper blocks are never computed.
"""

import os
import numpy as np
import ml_dtypes

import concourse.bass as bass
import concourse.tile as tile
from concourse import bacc, mybir
from concourse.bass_utils import run_bass_kernel_spmd

B, S, D, H = 2, 2048, 1024, 16
HD = D // H          # 64
HL = H // 4          # 4 heads per core
PL = HL * HD         # 256 local projection dim
KT = D // 128        # 8 contraction blocks
SB = S // 128        # 16 sequence blocks of 128
CH = S // 512        # 4 sequence chunks of 512
F32 = mybir.dt.float32
DT = mybir.dt.bfloat16
NP_DT = ml_dtypes.bfloat16

_cache = {}
last_results = None


def build_program():
    if "nc" in _cache:
        return _cache["nc"]
    nc = bacc.Bacc("TRN2", target_bir_lowering=False, debug=False, num_devices=8)

    qt_d = nc.dram_tensor("qt", [D, S], DT, kind="ExternalInput")
    kt_d = nc.dram_tensor("kt", [D, S], DT, kind="ExternalInput")
    vt_d = nc.dram_tensor("vt", [D, S], DT, kind="ExternalInput")
    wq_d = nc.dram_tensor("wq", [D, PL], DT, kind="ExternalInput")
    wk_d = nc.dram_tensor("wk", [D, PL], DT, kind="ExternalInput")
    wv_d = nc.dram_tensor("wv", [D, PL], DT, kind="ExternalInput")
    wf_d = nc.dram_tensor("wf", [PL, D], DT, kind="ExternalInput")
    bq_d = nc.dram_tensor("bq2", [2, 128, 1], F32, kind="ExternalInput")
    bk_d = nc.dram_tensor("bk2", [2, 128, 1], F32, kind="ExternalInput")
    bv_d = nc.dram_tensor("bv1", [1, PL], F32, kind="ExternalInput")
    bf_d = nc.dram_tensor("bf1", [1, D], F32, kind="ExternalInput")
    tri_d = nc.dram_tensor("tri", [128, 128], F32, kind="ExternalInput")
    sel_d = nc.dram_tensor("sel", [16, 16 * HD], mybir.dt.float16, kind="ExternalInput")
    out_d = nc.dram_tensor("out", [S, D], F32, kind="ExternalOutput")

    ADD = mybir.AluOpType.add
    MUL = mybir.AluOpType.mult
    EXP = mybir.ActivationFunctionType.Exp

    with tile.TileContext(nc) as tc:
        with (
            tc.tile_pool(name="singles", bufs=1) as singles,
            tc.tile_pool(name="inp", bufs=12) as inp,
            tc.tile_pool(name="epool", bufs=6) as epool,
            tc.tile_pool(name="apool", bufs=2) as apool,
            tc.tile_pool(name="opool", bufs=3) as opool,
            tc.tile_pool(name="psum", bufs=2, space="PSUM") as psum,
        ):
            wq_sb = singles.tile([128, KT, PL], DT)
            wk_sb = singles.tile([128, KT, PL], DT)
            wv_sb = singles.tile([128, KT, PL], DT)
            wf_sb = singles.tile([128, 2, D], DT)
            nc.sync.dma_start(wq_sb, wq_d.ap().rearrange("(k p) n -> p k n", p=128))
            nc.sync.dma_start(wk_sb, wk_d.ap().rearrange("(k p) n -> p k n", p=128))
            nc.sync.dma_start(wv_sb, wv_d.ap().rearrange("(k p) n -> p k n", p=128))
            nc.sync.dma_start(wf_sb, wf_d.ap().rearrange("(t p) n -> p t n", p=128))

            bq_sb = singles.tile([128, 2], F32)
            bk_sb = singles.tile([128, 2], F32)
            nc.sync.dma_start(bq_sb, bq_d.ap().rearrange("t p o -> p (t o)"))
            nc.sync.dma_start(bk_sb, bk_d.ap().rearrange("t p o -> p (t o)"))
            bv_bc = singles.tile([128, PL], F32)
            bf_bc = singles.tile([128, D], F32)
            nc.sync.dma_start(bv_bc, bv_d.ap().to_broadcast([128, PL]))
            nc.sync.dma_start(bf_bc, bf_d.ap().to_broadcast([128, D]))
            tri_sb = singles.tile([128, 128], F32)
            nc.sync.dma_start(tri_sb, tri_d.ap())

            qhT = singles.tile([128, 2, S], DT)   # [p within block, pblock, s]
            khT = singles.tile([128, 2, S], DT)
            vh = singles.tile([128, SB, HL, HD + 1], DT)  # [s within blk, sblk, h, hd|ones]
            xn = singles.tile([128, 2, S], DT)    # normalized attn out, head pairs stacked
            nc.vector.memset(vh[:, :, :, HD : HD + 1], 1.0)
            sel_sb = singles.tile([16, 16 * HD], mybir.dt.float16)
            nc.sync.dma_start(sel_sb, sel_d.ap())
            sums_sb = singles.tile([16, 512], F32)   # row hc = attn row-sums of (h, c)
            rsum16 = singles.tile([16, 512], mybir.dt.float16)

            # ---- Phase B: projections ----
            def load_blocks(x_d):
                ts = []
                for kk in range(KT):
                    t = inp.tile([128, S], DT, tag="xin")
                    nc.sync.dma_start(t, x_d.ap()[128 * kk : 128 * (kk + 1), :])
                    ts.append(t)
                return ts

            def proj_ph(xt, w_sb, b_sb, out_sb):
                for pt in range(2):
                    for ch in range(CH):
                        pp = psum.tile([128, 512], F32, tag="A", bufs=4)
                        for kk in range(KT):
                            nc.tensor.matmul(
                                pp,
                                lhsT=w_sb[:, kk, 128 * pt : 128 * (pt + 1)],
                                rhs=xt[kk][:, 512 * ch : 512 * (ch + 1)],
                                start=(kk == 0),
                                stop=(kk == KT - 1),
                            )
                        nc.vector.tensor_scalar_add(
                            out_sb[:, pt, 512 * ch : 512 * (ch + 1)],
                            pp,
                            b_sb[:, pt : pt + 1],
                        )

            qt_t = load_blocks(qt_d)
            proj_ph(qt_t, wq_sb, bq_sb, qhT)
            kt_t = load_blocks(kt_d)
            proj_ph(kt_t, wk_sb, bk_sb, khT)
            vt_t = load_blocks(vt_d)
            for sb in range(SB):
                pv = psum.tile([128, PL], F32, tag="C", bufs=2)
                for kk in range(KT):
                    nc.tensor.matmul(
                        pv,
                        lhsT=vt_t[kk][:, 128 * sb : 128 * (sb + 1)],
                        rhs=wv_sb[:, kk, :],
                        start=(kk == 0),
                        stop=(kk == KT - 1),
                    )
                nc.vector.tensor_tensor(
                    out=vh[:, sb, :, 0:HD],
                    in0=pv.rearrange("p (h e) -> p h e", h=HL),
                    in1=bv_bc.rearrange("p (h e) -> p h e", h=HL),
                    op=ADD,
                )

            # ---- Phase C: attention per head ----
            xas = []
            for h in range(HL):
                ro = 64 * (h % 2)
                hb = h // 2
                for c in range(CH):
                    px = psum.tile([128, 512], F32, tag="B", bufs=2)
                    nbj = 4 * c + 4
                    for bj in range(nbj):
                        band = bj >= 4 * c
                        i0 = 128 * bj if band else 512 * c
                        w = 512 * (c + 1) - i0
                        o = i0 - 512 * c
                        ps = psum.tile([128, 512], F32, tag="A", bufs=4)
                        nc.tensor.matmul(
                            ps[:, 0:w],
                            lhsT=khT[ro : ro + 64, hb, 128 * bj : 128 * (bj + 1)],
                            rhs=qhT[ro : ro + 64, hb, i0 : i0 + w],
                            start=True,
                            stop=True,
                        )
                        if band:
                            nc.vector.tensor_tensor(
                                out=ps[:, 0:128], in0=ps[:, 0:128], in1=tri_sb, op=ADD
                            )
                        et = epool.tile([128, 512], DT, tag="et")
                        nc.scalar.activation(et[:, 0:w], ps[:, 0:w], EXP)
                        nc.tensor.matmul(
                            px[0 : HD + 1, o : o + w],
                            lhsT=vh[:, bj, h, :],
                            rhs=et[:, 0:w],
                            start=(bj == 0),
                            stop=(bj == nbj - 1),
                        )
                    hc = 4 * h + c
                    xa = apool.tile([HD + 1, 512], F32, tag="xa", bufs=16)
                    nc.vector.tensor_copy(out=xa, in_=px[0 : HD + 1, :])
                    nc.sync.dma_start(sums_sb[hc : hc + 1, :], xa[HD : HD + 1, :])
                    xas.append(xa)

            # batched softmax renormalization: one reciprocal over all 16
            # (h, c) row-sum rows, then per-(h,c) partition-broadcast of the
            # reciprocal row via a one-hot fp16 selector matmul (gpsimd
            # partition_broadcast is broken on HW; step-0-partition DMA from
            # SBUF is rejected; single-partition DVE reciprocals cost 3.3us
            # each).
            nc.vector.reciprocal(sums_sb, sums_sb)
            nc.vector.tensor_copy(rsum16, sums_sb)
            for h in range(HL):
                ro = 64 * (h % 2)
                hb = h // 2
                for c in range(CH):
                    hc = 4 * h + c
                    rb = psum.tile([HD, 512], F32, tag="C", bufs=2)
                    nc.tensor.matmul(
                        rb,
                        lhsT=sel_sb[:, HD * hc : HD * (hc + 1)],
                        rhs=rsum16,
                        start=True,
                        stop=True,
                    )
                    xt_n = apool.tile([HD, 512], DT, tag="xtn")
                    nc.vector.tensor_tensor(
                        out=xt_n, in0=xas[hc][0:HD, :], in1=rb, op=MUL
                    )
                    nc.sync.dma_start(
                        xn[ro : ro + 64, hb, 512 * c : 512 * (c + 1)], xt_n
                    )

            # ---- Phase D: output projection (partial; host sums over groups) ----
            for ib in range(SB):
                for oc in range(2):
                    po = psum.tile([128, 512], F32, tag="A", bufs=4)
                    for t in range(2):
                        nc.tensor.matmul(
                            po,
                            lhsT=xn[:, t, 128 * ib : 128 * (ib + 1)],
                            rhs=wf_sb[:, t, 512 * oc : 512 * (oc + 1)],
                            start=(t == 0),
                            stop=(t == 1),
                        )
                    ob = opool.tile([128, 512], F32, tag="ob")
                    nc.vector.tensor_tensor(
                        out=ob, in0=po, in1=bf_bc[:, 512 * oc : 512 * (oc + 1)], op=ADD
                    )
                    nc.sync.dma_start(
                        out_d.ap()[128 * ib : 128 * (ib + 1), 512 * oc : 512 * (oc + 1)],
                        ob,
                    )

    nc.compile()
    _cache["nc"] = nc
    return nc


def make_in_maps(q, k, v, mask, Wq, bq, Wk, bk, Wv, bv, Wf, bf):
    scale = 1.0 / np.sqrt(np.float32(HD))
    f32 = np.float32
    m = np.asarray(mask[0, 0])
    tri = np.where(m[:128, :128].T == 0, f32(-1e9), f32(0.0)).astype(f32)
    sel = np.zeros((16, 16 * HD), np.float16)
    for hc in range(16):
        sel[hc, HD * hc : HD * (hc + 1)] = 1.0
    in_maps = []
    for c in range(8):
        b, g = c // 4, c % 4
        sl = slice(g * PL, (g + 1) * PL)
        in_maps.append(
            {
                "qt": np.ascontiguousarray((np.asarray(q[b]).T * scale)).astype(NP_DT),
                "kt": np.ascontiguousarray(np.asarray(k[b]).T).astype(NP_DT),
                "vt": np.ascontiguousarray(np.asarray(v[b]).T).astype(NP_DT),
                "wq": np.ascontiguousarray(np.asarray(Wq)[sl, :].T).astype(NP_DT),
                "wk": np.ascontiguousarray(np.asarray(Wk)[sl, :].T).astype(NP_DT),
                "wv": np.ascontiguousarray(np.asarray(Wv)[sl, :].T).astype(NP_DT),
                "wf": np.ascontiguousarray(np.asarray(Wf)[:, sl].T).astype(NP_DT),
                "bq2": (np.asarray(bq)[sl] * scale).astype(f32).reshape(2, 128, 1),
                "bk2": np.asarray(bk)[sl].astype(f32).reshape(2, 128, 1),
                "bv1": np.asarray(bv)[sl].astype(f32).reshape(1, PL),
                "bf1": (np.asarray(bf).astype(f32) / 4.0).reshape(1, D),
                "tri": tri,
                "sel": sel,
            }
        )
    return in_maps


def _mask_is_causal(mask):
    m = np.asarray(mask[0, 0])
    return bool(np.array_equal(m != 0, np.tril(np.ones((S, S), bool))))


def _numpy_fallback(q, k, v, mask, Wq, bq, Wk, bk, Wv, bv, Wf, bf):
    out = np.empty((B, S, D), np.float32)
    m = np.asarray(mask[0, 0])
    for b in range(B):
        qh = (np.asarray(q[b]) @ np.asarray(Wq).T + bq).reshape(S, H, HD)
        kh = (np.asarray(k[b]) @ np.asarray(Wk).T + bk).reshape(S, H, HD)
        vh = (np.asarray(v[b]) @ np.asarray(Wv).T + bv).reshape(S, H, HD)
        x = np.empty((S, H, HD), np.float32)
        for hh in range(H):
            sc = qh[:, hh] @ kh[:, hh].T / np.sqrt(np.float32(HD))
            sc = np.where(m == 0, np.float32(-1e9), sc)
            sc = sc - sc.max(-1, keepdims=True)
            e = np.exp(sc)
            x[:, hh] = (e / e.sum(-1, keepdims=True)) @ vh[:, hh]
        out[b] = x.reshape(S, D) @ np.asarray(Wf).T + bf
    return out


def kernel(q, k, v, mask, Wq, bq, Wk, bk, Wv, bv, Wf, bf):
    global last_results
    if not _mask_is_causal(mask):
        return _numpy_fallback(q, k, v, mask, Wq, bq, Wk, bk, Wv, bv, Wf, bf)
    nc = build_program()
    in_maps = make_in_maps(q, k, v, mask, Wq, bq, Wk, bk, Wv, bv, Wf, bf)
    res = run_bass_kernel_spmd(nc, in_maps, core_ids=list(range(8)))
    last_results = res
    out = np.zeros((B, S, D), np.float32)
    for c in range(8):
        out[c // 4] += res.results[c]["out"]
    return out
